# revision 1
# baseline (speedup 1.0000x reference)
"""COOTensorProduct kernel for 8 Trainium2 NeuronCores.

Math: out[b, h] = sum_{i,j} cb[h, i*64+j] * in1[b, i] * in2[b, j]
with in1/in2 [4096, 64], cb [4096, 4096] (a Clebsch-Gordan / Wigner-3j
coupling matrix for irreps '4x0e+4x1o+4x2e+4x3o' x same -> all l3).

cb is 0.1% dense but perfectly block-structured: for each (l1, l2) pair
of irrep types the coupling is a square (2l1+1)(2l2+1) x (2l1+1)(2l2+1)
matrix (stacked l3 blocks), identical across the 4x4 multiplicity copies
(u, v). The 16 pair matrices have sizes {1,3,3,5,5,7,7,9,15,15,21,21,
25,35,35,49} which pack block-diagonally into exactly two 128x128
stationary matrices (49+35+35+9 = 128 and the rest = 128).

Per core (512 batch rows):
  rhs[S][u,v]  = in1T_gathered[S,u] * in2T_gathered[S,v]   (elementwise,
                 [128 partitions = (pair,m1,m2) rows, 512 free = batch])
  psum[S][u,v] = W_S.T @ rhs        (one 128x128x512 matmul)
so the whole problem is 32 elementwise mults + 32 matmuls per core.

Host does the (static, index-only) gathers/permutes; device does all
FLOPs. Output comes back as [4096 permuted rows, 512 batch] per core and
is un-permuted/transposed on host.
"""

import json
import numpy as np

# ---------------------------------------------------------------- problem
B = 4096
DIM = 64
NCORES = 8
BPC = B // NCORES  # 512 batch rows per core
LMAX = 3
NMULT = 4  # multiplicity of each l in '4x0e+4x1o+4x2e+4x3o'
LS = [l for l in range(LMAX + 1) for _ in range(NMULT)]

# block-diagonal packing of the 16 (l1,l2) pair matrices into 2 stationaries
PAIRS_A = [(3, 3), (3, 2), (2, 3), (1, 1)]
PAIRS_B = [(2, 2), (1, 3), (3, 1), (1, 2), (2, 1), (0, 3), (3, 0),
           (0, 2), (2, 0), (0, 1), (1, 0), (0, 0)]

_decomp_cache = None
_nc_cache = None


def _col_start(l, u):
    return sum((2 * ll + 1) * NMULT for ll in range(l)) + u * (2 * l + 1)


def _build_decomp():
    """Index bookkeeping only (no numerics): which cb entries form the two
    stationary matrices, which in1/in2 columns feed each partition row,
    and which output row h each psum row maps to."""
    global _decomp_cache
    if _decomp_cache is not None:
        return _decomp_cache

    # replicate build_cb_matrix's row layout
    layout = {}
    idx1 = 0
    for l1 in LS:
        idx2 = 0
        for l2 in LS:
            for l3 in range(abs(l1 - l2), l1 + l2 + 1):
                layout.setdefault(l3, []).append((l1, l2, idx1 * DIM + idx2))
            idx2 += 2 * l2 + 1
        idx1 += 2 * l1 + 1
    entry_row = {}
    row = 0
    for l3 in sorted(layout):
        for (l1, l2, co) in sorted(layout[l3], key=lambda x: x[0] * LMAX + x[1]):
            entry_row[(l3, co)] = row
            row += 2 * l3 + 1
    assert row == B

    groups = []
    for pairs in (PAIRS_A, PAIRS_B):
        assert sum((2 * a + 1) * (2 * b + 1) for a, b in pairs) == 128
        c1 = np.zeros((NMULT, 128), dtype=np.int64)
        c2 = np.zeros((NMULT, 128), dtype=np.int64)
        h_of = np.zeros((NMULT, NMULT, 128), dtype=np.int64)
        w_k, w_m, w_h, w_c = [], [], [], []  # W[k,m] = cb[h, c]
        off = 0
        for (l1, l2) in pairs:
            n1, n2 = 2 * l1 + 1, 2 * l2 + 1
            kp = n1 * n2
            kk = np.arange(kp)
            m1, m2 = kk // n2, kk % n2
            for u in range(NMULT):
                c1[u, off:off + kp] = _col_start(l1, u) + m1
            for v in range(NMULT):
                c2[v, off:off + kp] = _col_start(l2, v) + m2
            mm = 0
            for l3 in range(abs(l1 - l2), l1 + l2 + 1):
                n3 = 2 * l3 + 1
                h0 = entry_row[(l3, _col_start(l1, 0) * DIM + _col_start(l2, 0))]
                km, m3m = np.meshgrid(kk, np.arange(n3), indexing="ij")
                w_k.append((off + km).ravel())
                w_m.append((off + mm + m3m).ravel())
                w_h.append((h0 + m3m).ravel())
                w_c.append(((_col_start(l1, 0) + m1[km.ravel()]) * DIM
                            + (_col_start(l2, 0) + m2[km.ravel()])))
                for u in range(NMULT):
                    for v in range(NMULT):
                        h = entry_row[(l3, _col_start(l1, u) * DIM + _col_start(l2, v))]
                        h_of[u, v, off + mm:off + mm + n3] = np.arange(h, h + n3)
                mm += n3
            off += kp
        groups.append({
            "c1": c1, "c2": c2, "h_of": h_of,
            "w_k": np.concatenate(w_k), "w_m": np.concatenate(w_m),
            "w_h": np.concatenate(w_h), "w_c": np.concatenate(w_c),
        })

    # global output row -> h map: tile t = S*16 + u*4 + v holds rows
    # t*128 + mm  ->  h_of[S][u, v, mm]
    hglob = np.zeros(32 * 128, dtype=np.int64)
    for s, g in enumerate(groups):
        for u in range(NMULT):
            for v in range(NMULT):
                t = s * 16 + u * 4 + v
                hglob[t * 128:(t + 1) * 128] = g["h_of"][u, v]
    _decomp_cache = (groups, hglob)
    return _decomp_cache


def _split_waits(bir_bytes):
    """This container's walrus build rejects >1 sync-wait per instruction
    ("Too many sync wait commands"). Hoist extra waits onto standalone
    EventSemaphore instructions on the same engine (same lowering raw
    bass wait_ge uses)."""
    bir = json.loads(bir_bytes)
    n = 0
    for fn in bir["functions"]:
        for blk in fn["blocks"]:
            out = []
            for inst in blk["instructions"]:
                si = inst.get("sync_info")
                waits = (si or {}).get("on_wait") or []
                if len(waits) > 1:
                    for w in waits[:-1]:
                        n += 1
                        out.append({
                            "debug": inst.get("debug", 0),
                            "engine": inst["engine"],
                            "ins": [], "outs": [],
                            "name": f"I-wsplit-{n}",
                            "opcode": "EventSemaphore",
                            "sync_info": {"on_update": [], "on_wait": [w]},
                        })
                    si["on_wait"] = [waits[-1]]
                out.append(inst)
            blk["instructions"] = out
    return json.dumps(bir).encode()


def _build_nc():
    """Bass program, identical on all 8 cores (SPMD; per-core data differs).

    Per core: 16 pre-gathered input tiles [128, 512] (partition = the
    (pair, m1, m2) rows of one packed stationary group, free = batch),
    32 elementwise products (DVE), 32 [128x128x512] matmuls against the
    two block-diagonal CG stationaries (PE), 32 PSUM->SBUF copies (ACT),
    32 output DMAs. Measured fastest of all variants profiled (fp32
    throughout; float32r / PE-side gather / fused-mult / packed-DMA /
    raw-pre-Tile-DMA variants all traced equal, slower, or incorrect).
    The kernel sits simultaneously near the fp32 PE floor (4 cyc/row),
    the DMA fabric floor (12.6 MB/core), and the DVE/ACT 1x elementwise
    floors.
    """
    global _nc_cache
    if _nc_cache is not None:
        return _nc_cache
    import concourse.bass as bass
    import concourse.mybir as mybir
    from concourse.tile import TileContext

    f32 = mybir.dt.float32
    nc = bass.Bass()
    w = nc.dram_tensor("w", [2, 128, 128], f32, kind="ExternalInput")
    g1 = nc.dram_tensor("g1", [8, 128, BPC], f32, kind="ExternalInput")
    g2 = nc.dram_tensor("g2", [8, 128, BPC], f32, kind="ExternalInput")
    o = nc.dram_tensor("o", [32, 128, BPC], f32, kind="ExternalOutput")

    with TileContext(nc) as tc:
        with (
            tc.tile_pool(name="wpool", bufs=1) as wpool,
            tc.tile_pool(name="gpool", bufs=1) as gpool,
            tc.tile_pool(name="rhspool", bufs=6) as rhspool,
            tc.tile_pool(name="psum", bufs=8, space="PSUM") as psumpool,
            tc.tile_pool(name="opool", bufs=8) as opool,
        ):
            wt = []
            for s in range(2):
                t = wpool.tile([128, 128], f32, tag=f"w{s}", name=f"w{s}")
                nc.sync.dma_start(out=t, in_=w[s, :, :])
                wt.append(t)
            g1t, g2t = [], []
            for i in range(8):
                t = gpool.tile([128, BPC], f32, tag=f"g1_{i}", name=f"g1_{i}")
                nc.sync.dma_start(out=t, in_=g1[i, :, :])
                g1t.append(t)
                t = gpool.tile([128, BPC], f32, tag=f"g2_{i}", name=f"g2_{i}")
                nc.sync.dma_start(out=t, in_=g2[i, :, :])
                g2t.append(t)

            for s in range(2):
                for u in range(NMULT):
                    for v in range(NMULT):
                        t = s * 16 + u * 4 + v
                        rhs = rhspool.tile([128, BPC], f32, tag="rhs")
                        nc.vector.tensor_mul(
                            out=rhs, in0=g1t[s * 4 + u], in1=g2t[s * 4 + v])
                        ps = psumpool.tile([128, BPC], f32, tag="ps")
                        nc.tensor.matmul(ps, wt[s], rhs, start=True, stop=True)
                        ot = opool.tile([128, BPC], f32, tag="ot")
                        if t >= 28:
                            # tail copies on DVE (its mults are done by now)
                            nc.vector.tensor_copy(out=ot, in_=ps)
                        else:
                            nc.scalar.copy(out=ot, in_=ps)
                        nc.sync.dma_start(out=o[t, :, :], in_=ot)

    orig = nc.to_json_bytes
    nc.to_json_bytes = lambda: _split_waits(orig())
    _nc_cache = nc
    return nc


def kernel(in1, in2, cb, _want_stats=False):
    from concourse.bass_utils import run_bass_kernel_spmd

    in1 = np.ascontiguousarray(np.asarray(in1, dtype=np.float32))
    in2 = np.ascontiguousarray(np.asarray(in2, dtype=np.float32))
    cb = np.asarray(cb, dtype=np.float32)
    groups, hglob = _build_decomp()

    # stationaries extracted straight from cb (no wigner math needed)
    wmat = np.zeros((2, 128, 128), dtype=np.float32)
    for s, g in enumerate(groups):
        wmat[s][g["w_k"], g["w_m"]] = cb[g["w_h"], g["w_c"]]

    in_maps = []
    for c in range(NCORES):
        sl = slice(c * BPC, (c + 1) * BPC)
        b1, b2 = in1[sl], in2[sl]
        gg1 = np.empty((8, 128, BPC), dtype=np.float32)
        gg2 = np.empty((8, 128, BPC), dtype=np.float32)
        for s, g in enumerate(groups):
            for u in range(NMULT):
                gg1[s * 4 + u] = b1.T[g["c1"][u]]
                gg2[s * 4 + u] = b2.T[g["c2"][u]]
        in_maps.append({"w": wmat, "g1": gg1, "g2": gg2})

    nc = _build_nc()
    import os
    trace = bool(int(os.environ.get("KERNEL_TRACE", "0")))
    res = run_bass_kernel_spmd(nc, in_maps, core_ids=list(range(NCORES)),
                               trace=trace)

    # [4096 permuted rows, 4096 batch]
    full = np.concatenate(
        [r["o"].reshape(32 * 128, BPC) for r in res.results], axis=1)
    out = np.empty((B, B), dtype=np.float32)
    out[:, hglob] = full.T
    if _want_stats:
        return out, res
    return out


if __name__ == "__main__":
    rng = np.random.default_rng(0)
    a = rng.standard_normal((B, DIM)).astype(np.float32)
    b = rng.standard_normal((B, DIM)).astype(np.float32)
    cb = np.load("/tmp/cb.npy")
    out = kernel(a, b, cb)
    outer = np.einsum("bi,bj->bij", a, b).reshape(B, -1)
    exp = outer @ cb.T
    print("rel err:", np.linalg.norm(out - exp) / np.linalg.norm(exp))



# revision 3
# speedup vs baseline: 1.3592x; 1.3592x over previous
"""COOTensorProduct kernel for 8 Trainium2 NeuronCores (bf16 pipeline).

Math: out[b, h] = sum_{i,j} cb[h, i*64+j] * in1[b, i] * in2[b, j]
with in1/in2 [4096, 64], cb [4096, 4096] (Clebsch-Gordan coupling for
irreps '4x0e+4x1o+4x2e+4x3o' x same -> all l3).

cb is block-structured: the 16 (l1,l2) pair couplings are square
matrices that pack block-diagonally into two 128x128 stationaries.
Per core (512 batch rows), for each group s and multiplicity pair
(u, v): rhs = g1[s,u] * g2[s,v] elementwise, psum = W_s.T @ rhs.

This version runs the whole device pipeline in bf16 (rel-err budget is
2e-2; bf16 end-to-end measures ~2e-3): 1-pass matmuls instead of the
fp32 LOW/HIGH dual pass, half the DMA bytes, 2x DVE rate. Ops are
batched 4-wide over v ([128, 2048] supertiles; PSUM tiles span 4
banks) to cut instruction/semaphore count, input DMAs ride the scalar
HWDGE ring while outputs ride the sync ring, and the PSUM->SBUF
drains are spread over scalar/gpsimd/vector.
"""

import json
import numpy as np
import ml_dtypes

BF16 = ml_dtypes.bfloat16

# ---------------------------------------------------------------- problem
B = 4096
DIM = 64
NCORES = 8
BPC = B // NCORES  # 512 batch rows per core
LMAX = 3
NMULT = 4  # multiplicity of each l in '4x0e+4x1o+4x2e+4x3o'
LS = [l for l in range(LMAX + 1) for _ in range(NMULT)]

# block-diagonal packing of the 16 (l1,l2) pair matrices into 2 stationaries
PAIRS_A = [(3, 3), (3, 2), (2, 3), (1, 1)]
PAIRS_B = [(2, 2), (1, 3), (3, 1), (1, 2), (2, 1), (0, 3), (3, 0),
           (0, 2), (2, 0), (0, 1), (1, 0), (0, 0)]

_decomp_cache = None
_nc_cache = None


def _col_start(l, u):
    return sum((2 * ll + 1) * NMULT for ll in range(l)) + u * (2 * l + 1)


def _build_decomp():
    """Index bookkeeping only (no numerics): which cb entries form the two
    stationary matrices, which in1/in2 columns feed each partition row,
    and which output row h each psum row maps to."""
    global _decomp_cache
    if _decomp_cache is not None:
        return _decomp_cache

    # replicate build_cb_matrix's row layout
    layout = {}
    idx1 = 0
    for l1 in LS:
        idx2 = 0
        for l2 in LS:
            for l3 in range(abs(l1 - l2), l1 + l2 + 1):
                layout.setdefault(l3, []).append((l1, l2, idx1 * DIM + idx2))
            idx2 += 2 * l2 + 1
        idx1 += 2 * l1 + 1
    entry_row = {}
    row = 0
    for l3 in sorted(layout):
        for (l1, l2, co) in sorted(layout[l3], key=lambda x: x[0] * LMAX + x[1]):
            entry_row[(l3, co)] = row
            row += 2 * l3 + 1
    assert row == B

    groups = []
    for pairs in (PAIRS_A, PAIRS_B):
        assert sum((2 * a + 1) * (2 * b + 1) for a, b in pairs) == 128
        c1 = np.zeros((NMULT, 128), dtype=np.int64)
        c2 = np.zeros((NMULT, 128), dtype=np.int64)
        h_of = np.zeros((NMULT, NMULT, 128), dtype=np.int64)
        w_k, w_m, w_h, w_c = [], [], [], []  # W[k,m] = cb[h, c]
        off = 0
        for (l1, l2) in pairs:
            n1, n2 = 2 * l1 + 1, 2 * l2 + 1
            kp = n1 * n2
            kk = np.arange(kp)
            m1, m2 = kk // n2, kk % n2
            for u in range(NMULT):
                c1[u, off:off + kp] = _col_start(l1, u) + m1
            for v in range(NMULT):
                c2[v, off:off + kp] = _col_start(l2, v) + m2
            mm = 0
            for l3 in range(abs(l1 - l2), l1 + l2 + 1):
                n3 = 2 * l3 + 1
                h0 = entry_row[(l3, _col_start(l1, 0) * DIM + _col_start(l2, 0))]
                km, m3m = np.meshgrid(kk, np.arange(n3), indexing="ij")
                w_k.append((off + km).ravel())
                w_m.append((off + mm + m3m).ravel())
                w_h.append((h0 + m3m).ravel())
                w_c.append(((_col_start(l1, 0) + m1[km.ravel()]) * DIM
                            + (_col_start(l2, 0) + m2[km.ravel()])))
                for u in range(NMULT):
                    for v in range(NMULT):
                        h = entry_row[(l3, _col_start(l1, u) * DIM + _col_start(l2, v))]
                        h_of[u, v, off + mm:off + mm + n3] = np.arange(h, h + n3)
                mm += n3
            off += kp
        groups.append({
            "c1": c1, "c2": c2, "h_of": h_of,
            "w_k": np.concatenate(w_k), "w_m": np.concatenate(w_m),
            "w_h": np.concatenate(w_h), "w_c": np.concatenate(w_c),
        })

    # global output row -> h map: tile t = S*16 + u*4 + v holds rows
    # t*128 + mm  ->  h_of[S][u, v, mm]
    hglob = np.zeros(32 * 128, dtype=np.int64)
    for s, g in enumerate(groups):
        for u in range(NMULT):
            for v in range(NMULT):
                t = s * 16 + u * 4 + v
                hglob[t * 128:(t + 1) * 128] = g["h_of"][u, v]
    _decomp_cache = (groups, hglob)
    return _decomp_cache


def _split_waits(bir_bytes):
    """This container's walrus build rejects >1 sync-wait per instruction
    ("Too many sync wait commands"). Hoist extra waits onto standalone
    EventSemaphore instructions on the same engine (same lowering raw
    bass wait_ge uses)."""
    bir = json.loads(bir_bytes)
    n = 0
    for fn in bir["functions"]:
        for blk in fn["blocks"]:
            out = []
            for inst in blk["instructions"]:
                si = inst.get("sync_info")
                waits = (si or {}).get("on_wait") or []
                if len(waits) > 1:
                    for w in waits[:-1]:
                        n += 1
                        out.append({
                            "debug": inst.get("debug", 0),
                            "engine": inst["engine"],
                            "ins": [], "outs": [],
                            "name": f"I-wsplit-{n}",
                            "opcode": "EventSemaphore",
                            "sync_info": {"on_update": [], "on_wait": [w]},
                        })
                    si["on_wait"] = [waits[-1]]
                out.append(inst)
            blk["instructions"] = out
    return json.dumps(bir).encode()


def _build_nc():
    """Bass program, identical on all 8 cores (SPMD; per-core data differs).

    Per core: 4 input supertiles [128, 2048] bf16 (one per (input, s),
    u/v-major), 8 DVE products ([128, 2048], in0 broadcast over v), 32
    bf16 [128x128x512] matmuls into 4-bank PSUM supertiles, 8 batched
    PSUM->SBUF bf16 copies (scalar/gpsimd/vector), 8 output DMAs (sync
    ring) with input DMAs on the scalar HWDGE ring.
    """
    global _nc_cache
    if _nc_cache is not None:
        return _nc_cache
    import concourse.bass as bass
    import concourse.mybir as mybir
    from concourse.tile import TileContext

    bf16 = mybir.dt.bfloat16
    f32 = mybir.dt.float32
    nc = bass.Bass()
    w = nc.dram_tensor("w", [2, 128, 128], bf16, kind="ExternalInput")
    g1 = nc.dram_tensor("g1", [2, 128, 4 * BPC], bf16, kind="ExternalInput")
    g2 = nc.dram_tensor("g2", [2, 128, 4 * BPC], bf16, kind="ExternalInput")
    o = nc.dram_tensor("o", [8, 128, 4 * BPC], bf16, kind="ExternalOutput")

    with TileContext(nc) as tc:
        with (
            tc.tile_pool(name="wpool", bufs=1) as wpool,
            tc.tile_pool(name="gpool", bufs=1) as gpool,
            tc.tile_pool(name="rhspool", bufs=3) as rhspool,
            tc.tile_pool(name="psum", bufs=2, space="PSUM") as psumpool,
            tc.tile_pool(name="opool", bufs=3) as opool,
        ):
            wt = []
            for s in range(2):
                t = wpool.tile([128, 128], bf16, tag=f"w{s}", name=f"w{s}")
                nc.scalar.dma_start(out=t, in_=w[s, :, :])
                wt.append(t)
            g1t, g2t = [], []
            for s in range(2):
                # s=0 inputs on the scalar HWDGE ring (needed first),
                # s=1 on the gpsimd SWDGE ring so both load concurrently.
                ring = nc.scalar if s == 0 else nc.gpsimd
                t = gpool.tile([128, 4 * BPC], bf16, tag=f"g1_{s}", name=f"g1_{s}")
                ring.dma_start(out=t, in_=g1[s, :, :])
                g1t.append(t)
                t = gpool.tile([128, 4 * BPC], bf16, tag=f"g2_{s}", name=f"g2_{s}")
                ring.dma_start(out=t, in_=g2[s, :, :])
                g2t.append(t)

            # psum->sbuf drain engine per (s, u): gpsimd has no PSUM
            # access, so spread the 8 batched copies over scalar/vector.
            copy_eng = {}
            for s in range(2):
                for u in range(NMULT):
                    st = s * 4 + u
                    copy_eng[st] = "scalar" if u in (0, 2) else "vector"

            for s in range(2):
                for u in range(NMULT):
                    st = s * 4 + u
                    rhs = rhspool.tile([128, 4 * BPC], bf16, tag="rhs")
                    # in0: g1 u-slice broadcast across the 4 v quarters
                    in0 = g1t[s][:, u * BPC:(u + 1) * BPC]
                    in0 = in0.unsqueeze(1).broadcast_to([128, 4, BPC])
                    in1 = g2t[s].rearrange("p (v c) -> p v c", v=4)
                    nc.vector.tensor_mul(
                        out=rhs.rearrange("p (v c) -> p v c", v=4),
                        in0=in0, in1=in1)
                    ps = psumpool.tile([128, 4 * BPC], f32, tag="ps")
                    for v in range(NMULT):
                        nc.tensor.matmul(
                            ps[:, v * BPC:(v + 1) * BPC], wt[s],
                            rhs[:, v * BPC:(v + 1) * BPC],
                            start=True, stop=True)
                    ot = opool.tile([128, 4 * BPC], bf16, tag="ot")
                    eng = getattr(nc, copy_eng[st])
                    if copy_eng[st] == "scalar":
                        eng.copy(out=ot, in_=ps)
                    else:
                        eng.tensor_copy(out=ot, in_=ps)
                    nc.sync.dma_start(out=o[st, :, :], in_=ot)

    orig = nc.to_json_bytes
    nc.to_json_bytes = lambda: _split_waits(orig())
    _nc_cache = nc
    return nc


def kernel(in1, in2, cb, _want_stats=False):
    from concourse.bass_utils import run_bass_kernel_spmd

    in1 = np.ascontiguousarray(np.asarray(in1, dtype=np.float32))
    in2 = np.ascontiguousarray(np.asarray(in2, dtype=np.float32))
    cb = np.asarray(cb, dtype=np.float32)
    groups, hglob = _build_decomp()

    # stationaries extracted straight from cb (no wigner math needed)
    wmat = np.zeros((2, 128, 128), dtype=np.float32)
    for s, g in enumerate(groups):
        wmat[s][g["w_k"], g["w_m"]] = cb[g["w_h"], g["w_c"]]
    wmat = wmat.astype(BF16)

    in_maps = []
    for c in range(NCORES):
        sl = slice(c * BPC, (c + 1) * BPC)
        b1 = in1[sl].T.astype(BF16)
        b2 = in2[sl].T.astype(BF16)
        gg1 = np.empty((2, 128, 4 * BPC), dtype=BF16)
        gg2 = np.empty((2, 128, 4 * BPC), dtype=BF16)
        for s, g in enumerate(groups):
            for u in range(NMULT):
                gg1[s][:, u * BPC:(u + 1) * BPC] = b1[g["c1"][u]]
                gg2[s][:, u * BPC:(u + 1) * BPC] = b2[g["c2"][u]]
        in_maps.append({"w": wmat, "g1": gg1, "g2": gg2})

    nc = _build_nc()
    import os
    trace = bool(int(os.environ.get("KERNEL_TRACE", "0")))
    res = run_bass_kernel_spmd(nc, in_maps, core_ids=list(range(NCORES)),
                               trace=trace)

    # o [8, 128, 2048]: supertile st = s*4+u, quarter v -> tile t = st*4+v
    full = np.concatenate(
        [np.asarray(r["o"], dtype=np.float32)
         .reshape(8, 128, 4, BPC).transpose(0, 2, 1, 3).reshape(32 * 128, BPC)
         for r in res.results], axis=1)
    out = np.empty((B, B), dtype=np.float32)
    out[:, hglob] = full.T
    if _want_stats:
        return out, res
    return out


if __name__ == "__main__":
    rng = np.random.default_rng(0)
    a = rng.standard_normal((B, DIM)).astype(np.float32)
    b = rng.standard_normal((B, DIM)).astype(np.float32)
    cb = np.load("/tmp/cb.npy")
    out = kernel(a, b, cb)
    outer = np.einsum("bi,bj->bij", a, b).reshape(B, -1)
    exp = outer @ cb.T
    print("rel err:", np.linalg.norm(out - exp) / np.linalg.norm(exp))
